# revision 4
# baseline (speedup 1.0000x reference)
# Mistral sliding-window attention (B=1, S=2048, H=4096, 32 q heads / 8 kv
# heads, window 4096 -> plain causal at this S) on 8 Trainium2 NeuronCores.
#
# Sharding: tensor-parallel over heads with NO on-device collectives. Core c
# owns q heads 4c..4c+3 and kv head c. hidden_states is replicated
# (transposed on host to [H, S]). Each core computes attention for its 4
# heads and then a PARTIAL o_proj over the FULL 4096 output columns using
# only its own 512 attention dims; the host sums the 8 partial outputs.
#
# v3 (vs baseline): the QKV projection runs a HYBRID chain per token chunk
# on a 7-bank PSUM rotation: 6-wide over k-tiles 0..15 (so the startup
# only demands ~250GB/s of x/w supply instead of ~500 and HAM warms
# early), then 3-wide m0..m2 over k 16..31, whose stage+rope epilogue
# overlaps the 3-wide m3..m5 tail; the first k-tiles of w/x are
# partition-sliced across DMA engines so the first matmul issues early;
# each next chunk's first x-group is prefetched double-buffered; V-tile
# PE transposes are deferred into the next chunk's chain; attention
# score / o_proj chains rotate over 4 PSUM banks instead of 3; and the
# o_proj psum->sbuf staging copies run 3:1 on vector:scalar.

from contextlib import ExitStack

import numpy as np
import ml_dtypes

import concourse.bacc as bacc
import concourse.bass as bass
import concourse.mybir as mybir
import concourse.tile as tile
from concourse.bass_utils import run_bass_kernel_spmd
from concourse.masks import make_identity

HIDDEN = 4096
NH = 32
NKV = 8
HD = 128
THETA = 10000.0
S = 2048
NCORES = 8

QH = NH // NCORES          # 4 q heads per core
DQ = QH * HD               # 512 (per-core attn width)
DOUT = DQ + 2 * HD         # 768 = q heads + k + v projection width
MT = DOUT // 128           # 6 projection m-tiles (0..3 q, 4 k, 5 v)
KT = HIDDEN // 128         # 32 contraction tiles
KG = 4                     # x-load group: k-tiles per DMA
KH = 16                    # hybrid switch: 6-wide below, 3+3 above
TCH = 512                  # token chunk (matmul moving dim)
NTCH = S // TCH            # 4
KVT = S // 128             # 16 kv tiles
HG = HIDDEN // TCH         # 8 o_proj output column groups
SCALE = 1.0 / float(np.sqrt(HD))

F32 = mybir.dt.float32
BF16 = mybir.dt.bfloat16
EXP = mybir.ActivationFunctionType.Exp

ROT = [f"r{i}" for i in range(7)]   # proj psum rotation (7 banks)
SCT = ["r4", "r5", "r6", "pvt"]     # attention scores / o_proj chains


def _rope(nc, pool, src, dst, cs, sn):
    """dst = src*cos + rotate_half(src)*sin, in [d, tok] layout.

    src/dst are [128, n]. cs is cos duplicated into both 64-row halves;
    sn is [+sin; -sin]. Then with B = src*sn and Bx = halves-swapped B:
    dst = src*cs + Bx reproduces rope with only whole-tile (partition-
    aligned) DVE ops plus two small SBUF->SBUF swap DMAs.
    """
    A = pool.tile([128, TCH], F32, name="ropeA")
    B = pool.tile([128, TCH], F32, name="ropeB")
    Bx = pool.tile([128, TCH], F32, name="ropeBx")
    nc.vector.tensor_mul(A, src, cs)
    nc.vector.tensor_mul(B, src, sn)
    # swaps ride the idle gpsimd SWDGE queue so they delay neither the
    # sync queue's x-group/output issues nor the scalar FIFO's staging
    nc.gpsimd.dma_start(out=Bx[0:64, :], in_=B[64:128, :])
    nc.gpsimd.dma_start(out=Bx[64:128, :], in_=B[0:64, :])
    nc.vector.tensor_add(dst, A, Bx)


def build_kernel_body(ctx: ExitStack, tc: tile.TileContext, outs, ins):
    nc = tc.nc
    xT, wqkv, ow, cos_t, sin_t, stair = (
        ins["xT"], ins["wqkv"], ins["ow"], ins["cos_t"], ins["sin_t"], ins["stair"],
    )
    out = outs["out"]

    # ONE pool scope for the whole kernel: closing a pool between phases
    # emits an all-engine barrier. PSUM (8 banks) is tag-partitioned:
    #   r0..r6 : proj 6 chains per chunk rotate over these 7 banks; in the
    #            attention phase r0/r1 = P@V psums, r2/r3 = denominators,
    #            r4..r6 = score tiles / o_proj chains
    #   pvt    : V transposes in proj, 4th score/o_proj bank in attention
    singles = ctx.enter_context(tc.tile_pool(name="singles", bufs=1))
    wp = ctx.enter_context(tc.tile_pool(name="wq", bufs=1))
    xp = ctx.enter_context(tc.tile_pool(name="xt", bufs=1))
    rps = ctx.enter_context(tc.tile_pool(name="ropes", bufs=1))
    rpv = ctx.enter_context(tc.tile_pool(name="ropev", bufs=2))
    ptp = ctx.enter_context(tc.tile_pool(name="pt", bufs=16))
    aop = ctx.enter_context(tc.tile_pool(name="ao", bufs=8))
    rcp = ctx.enter_context(tc.tile_pool(name="rc", bufs=2))
    obp = ctx.enter_context(tc.tile_pool(name="ob", bufs=4))
    pq = ctx.enter_context(tc.tile_pool(name="pq", bufs=1, space="PSUM"))

    stair_sb = singles.tile([128, 896], BF16)
    ones_sb = singles.tile([128, 128], BF16)
    ow_sb = singles.tile([128, QH, HIDDEN], BF16)   # [d, head, hid]
    qTc = [singles.tile([128, QH, TCH], BF16, name=f"qT{t}")
           for t in range(NTCH)]
    kTc = [singles.tile([128, TCH], BF16, name=f"kT{t}")
           for t in range(NTCH)]
    Vc = [singles.tile([128, 4, HD], BF16, name=f"V{t}")
          for t in range(NTCH)]                     # Vc[t][:, j%4, :] = [tok, d]

    cos_sb = wp.tile([128, S], BF16)
    sin_sb = wp.tile([128, S], BF16)
    vT = wp.tile([128, S], BF16)
    ident_sb = wp.tile([128, 128], BF16)
    nc.vector.memset(ones_sb, 1.0)
    make_identity(nc, ident_sb)

    # ---- phase 1: QKV projection + RoPE --------------------------------
    wq3 = wqkv.rearrange("(a p) d -> p a d", p=128)   # [128, KT, DOUT]
    x3 = xT.rearrange("(k p) s -> p k s", p=128)
    NG = KT // KG                                     # 8 x-load groups
    w_sb = [wp.tile([128, DOUT], BF16, name=f"w{k}", tag=f"w{k}")
            for k in range(KT)]

    # t=0 x tiles: group 0 is double-buffered (prefetched for t>0)
    def _xg0_tile(t):
        return xp.tile([128, KG, TCH], BF16, name="xg0", tag=f"xg0{t % 2}")

    xg_first = _xg0_tile(0)
    xgs_t0 = [xg_first] + [
        xp.tile([128, KG, TCH], BF16, name="xg", tag=f"xg{g}")
        for g in range(1, NG)
    ]
    # The dma_start ISSUE rate (~640ns each on the sync queue) bounds the
    # startup supply, so the first tiles use few, k-ordered DMAs: per-k w
    # tiles and x k-tile PAIRS for k 0..7, then group DMAs. cos/sin are
    # deferred behind the first half of the supply (needed only at the
    # first rope, ~45us in).
    for kp in range(4):
        k0 = 2 * kp
        nc.sync.dma_start(out=w_sb[k0], in_=wq3[:, k0, :])
        nc.sync.dma_start(
            out=xgs_t0[kp // 2][:, (k0 % KG):(k0 % KG) + 2, :],
            in_=x3[:, k0:k0 + 2, 0:TCH])
        nc.sync.dma_start(out=w_sb[k0 + 1], in_=wq3[:, k0 + 1, :])
        if kp == 1:
            # group 2's x pulled forward so it lands before k8 is consumed
            nc.sync.dma_start(out=xgs_t0[2], in_=x3[:, 8:12, 0:TCH])
    for k in range(8, 12):
        nc.sync.dma_start(out=w_sb[k], in_=wq3[:, k, :])
    for kg in range(3, NG):
        for k in range(kg * KG, (kg + 1) * KG):
            nc.sync.dma_start(out=w_sb[k], in_=wq3[:, k, :])
        nc.sync.dma_start(out=xgs_t0[kg],
                          in_=x3[:, kg * KG:(kg + 1) * KG, 0:TCH])
        if kg == 4:
            nc.sync.dma_start(out=cos_sb, in_=cos_t)
            nc.sync.dma_start(out=sin_sb, in_=sin_t)

    # PE warm-up: the HAM clock gate holds the PE at 1.2GHz until it has
    # been busy ~3.4us. The first x/w tiles only land ~9us in, so spin
    # small matmuls on ones_sb into the (otherwise unused until t=1) r6
    # bank; the real chunk-0 chain then starts at full 2.4GHz.
    warm = pq.tile([128, 128], F32, name="warm", tag="r6")
    for _ in range(44):
        nc.tensor.matmul(warm, lhsT=ones_sb, rhs=ones_sb,
                         start=True, stop=True)

    # ---- projection chunks: hybrid 6-wide head, 3+3 staggered tail -----
    xg0_next = None
    pending_vt = [None]

    def _emit_vt():
        if pending_vt[0] is not None:
            tv = pending_vt[0]
            pending_vt[0] = None
            for j in range(4):
                jj = 4 * tv + j
                pv = pq.tile([128, 128], BF16, name="pvt", tag="pvt")
                nc.tensor.transpose(pv, vT[:, jj * 128:(jj + 1) * 128],
                                    ident_sb)
                nc.scalar.copy(out=Vc[tv][:, j, :], in_=pv)

    for t in range(NTCH):
        if t == 0:
            xgs = xgs_t0
        else:
            xgs = [xg0_next]        # prefetched during chunk t-1
            for kg in range(1, NG):
                xg = xp.tile([128, KG, TCH], BF16, name="xg", tag=f"xg{kg}")
                nc.sync.dma_start(
                    out=xg,
                    in_=x3[:, kg * KG:(kg + 1) * KG, t * TCH:(t + 1) * TCH])
                xgs.append(xg)
        cs = cos_sb[:, t * TCH:(t + 1) * TCH]
        sn = sin_sb[:, t * TCH:(t + 1) * TCH]
        pst = [pq.tile([128, TCH], F32, name=f"ps{mi}",
                       tag=ROT[(6 * t + mi) % 7]) for mi in range(MT)]
        # 6-wide over k-tiles 0..KH-1
        for kg in range(KH // KG):
            for ki in range(KG):
                k = kg * KG + ki
                for mi in range(MT):
                    nc.tensor.matmul(
                        pst[mi],
                        lhsT=w_sb[k][:, mi * 128:(mi + 1) * 128],
                        rhs=xgs[kg][:, ki, :],
                        start=(k == 0), stop=False,
                    )
            if kg == 0:
                _emit_vt()          # prev chunk's V transposes
            if kg == 1 and t + 1 < NTCH:
                # prefetch next chunk's first x-group
                xg0_next = _xg0_tile(t + 1)
                nc.sync.dma_start(
                    out=xg0_next,
                    in_=x3[:, 0:KG, (t + 1) * TCH:(t + 2) * TCH])
        # 3-wide m0..m2 tail, then its epilogue overlaps the m3..m5 tail
        for k in range(KH, KT):
            for mi in range(3):
                nc.tensor.matmul(
                    pst[mi],
                    lhsT=w_sb[k][:, mi * 128:(mi + 1) * 128],
                    rhs=xgs[k // KG][:, k % KG, :],
                    start=False, stop=(k == KT - 1),
                )
        for mi in range(3):
            stg = rps.tile([128, TCH], F32, name="stg", tag=f"sg{mi}")
            nc.scalar.copy(out=stg, in_=pst[mi])
            _rope(nc, rpv, stg, qTc[t][:, mi, :], cs, sn)
        for k in range(KH, KT):
            for mi in range(3, MT):
                nc.tensor.matmul(
                    pst[mi],
                    lhsT=w_sb[k][:, mi * 128:(mi + 1) * 128],
                    rhs=xgs[k // KG][:, k % KG, :],
                    start=False, stop=(k == KT - 1),
                )
        stg = rps.tile([128, TCH], F32, name="stg", tag="sg0")
        nc.scalar.copy(out=stg, in_=pst[3])
        _rope(nc, rpv, stg, qTc[t][:, 3, :], cs, sn)
        stg2 = rps.tile([128, TCH], F32, name="stg", tag="sg1")
        nc.scalar.copy(out=stg2, in_=pst[4])
        _rope(nc, rpv, stg2, kTc[t], cs, sn)
        nc.scalar.copy(out=vT[:, t * TCH:(t + 1) * TCH], in_=pst[5])
        pending_vt[0] = t
        if t == 0:
            nc.sync.dma_start(out=stair_sb, in_=stair)
        elif t == 1:
            # 4MB ow load deferred here (first needed ~200us in) so it
            # does not compete with chunk 1's x supply
            nc.sync.dma_start(out=ow_sb, in_=ow)
    _emit_vt()                      # t=3's V transposes

    # ---- phase 2: attention + partial o_proj, software-pipelined -------
    def attn_S(h, c):
        """Scores + exp + causal stair; diagonal tiles only compute the
        unmasked q-column range."""
        pts = []
        qslice = qTc[c][:, h, :]
        for j in range(4 * c + 4):
            rdiag = j - 4 * c
            q0 = rdiag * 128 if rdiag > 0 else 0
            sc = pq.tile([128, TCH], F32, name="sc", tag=SCT[j % 4])
            nc.tensor.matmul(sc[:, q0:],
                             lhsT=kTc[j // 4][:, (j % 4) * 128:
                                              (j % 4 + 1) * 128],
                             rhs=qslice[:, q0:], start=True, stop=True)
            pt = ptp.tile([128, TCH], BF16, name="pt", tag="pt")
            nc.scalar.activation(pt[:, q0:], sc[:, q0:], EXP, scale=SCALE)
            if rdiag >= 0:  # tile touches the causal diagonal
                nc.vector.tensor_mul(pt[:, q0:], pt[:, q0:],
                                     stair_sb[:, 384:384 + TCH - q0])
            pts.append((pt, q0))
        return pts

    def attn_PV(u, h, c, pts):
        """P@V + denominator + normalize for one (head, q-chunk)."""
        jmax = 4 * c + 3
        po = pq.tile([128, TCH], F32, name="po", tag=ROT[u % 2])
        den = pq.tile([128, TCH], F32, name="den", tag=ROT[2 + u % 2])
        for j, (pt, q0) in enumerate(pts):
            nc.tensor.matmul(po[:, q0:], lhsT=Vc[j // 4][:, j % 4, :],
                             rhs=pt[:, q0:],
                             start=(j == 0), stop=(j == jmax))
            nc.tensor.matmul(den[:, q0:], lhsT=ones_sb, rhs=pt[:, q0:],
                             start=(j == 0), stop=(j == jmax))
        rec = rcp.tile([128, TCH], F32, name="rec")
        nc.vector.reciprocal_approx_fast(rec, den)
        ao = aop.tile([128, TCH], BF16, name="ao")
        nc.vector.tensor_mul(ao, po, rec)
        return ao

    def oproj(c, aos):
        """Partial o_proj for token chunk c: out[tok, :] over all 4096
        columns, contracting this core's 4 heads (512 attn dims)."""
        idx = 0
        for ts in range(TCH // 128):
            for hg in range(HG):
                op = pq.tile([128, TCH], F32, name="op", tag=SCT[idx % 4])
                for h in range(QH):
                    nc.tensor.matmul(
                        op,
                        lhsT=aos[h][:, ts * 128:(ts + 1) * 128],
                        rhs=ow_sb[:, h, hg * TCH:(hg + 1) * TCH],
                        start=(h == 0), stop=(h == QH - 1),
                    )
                ob = obp.tile([128, TCH], BF16, name="ob")
                # staging copies 3:1 on the two fast engines
                if idx % 4 == 3:
                    nc.scalar.copy(out=ob, in_=op)
                else:
                    nc.vector.tensor_copy(ob, op)
                idx += 1
                r0 = c * TCH + ts * 128
                nc.sync.dma_start(
                    out=out[r0:r0 + 128, hg * TCH:(hg + 1) * TCH], in_=ob)

    # chunk order: c=1 first (dense enough to keep HAM warm, inputs long
    # ready — c=3's q/k/V are still in the rope pipeline right after the
    # proj phase), c=0 last (its o_proj tail is the same size as any).
    units = [(c, h) for c in (1, 2, 3, 0) for h in range(QH)]
    pts_cur = attn_S(units[0][1], units[0][0])
    aos = []
    for idx, (c, h) in enumerate(units):
        if idx + 1 < len(units):
            c2, h2 = units[idx + 1]
            pts_nxt = attn_S(h2, c2)
        else:
            pts_nxt = None
        aos.append(attn_PV(idx, h, c, pts_cur))
        pts_cur = pts_nxt
        if h == QH - 1:
            oproj(c, aos)
            aos = []


_NC_CACHE = None


def build_program():
    global _NC_CACHE
    if _NC_CACHE is not None:
        return _NC_CACHE
    nc = bacc.Bacc("TRN2", target_bir_lowering=False, debug=False,
                   num_devices=NCORES)
    ins = {
        "xT": nc.dram_tensor("xT", [HIDDEN, S], BF16, kind="ExternalInput").ap(),
        "wqkv": nc.dram_tensor("wqkv", [HIDDEN, DOUT], BF16,
                               kind="ExternalInput").ap(),
        "ow": nc.dram_tensor("ow", [128, QH, HIDDEN], BF16,
                             kind="ExternalInput").ap(),
        "cos_t": nc.dram_tensor("cos_t", [128, S], BF16,
                                kind="ExternalInput").ap(),
        "sin_t": nc.dram_tensor("sin_t", [128, S], BF16,
                                kind="ExternalInput").ap(),
        "stair": nc.dram_tensor("stair", [128, 896], BF16,
                                kind="ExternalInput").ap(),
    }
    outs = {"out": nc.dram_tensor("out", [S, HIDDEN], BF16,
                                  kind="ExternalOutput").ap()}
    with tile.TileContext(nc) as tc:
        with ExitStack() as ctx:
            build_kernel_body(ctx, tc, outs, ins)
    nc.compile()
    _NC_CACHE = nc
    return nc


def make_in_maps(hidden_states, position_ids, q_w, k_w, v_w, o_w):
    bf16 = ml_dtypes.bfloat16
    x = np.asarray(hidden_states, dtype=np.float32).reshape(S, HIDDEN)
    xT = np.ascontiguousarray(x.T).astype(bf16)
    pos = np.asarray(position_ids).reshape(S).astype(np.float64)
    inv = 1.0 / (THETA ** (np.arange(0, HD, 2, dtype=np.float64) / HD))
    fr = inv[:, None] * pos[None, :]                       # [64, S]
    # cos duplicated into both 64-row halves; sin stored [+sin; -sin] so
    # rope becomes whole-tile muls + a halves-swap (see _rope)
    c64 = np.cos(fr)
    s64 = np.sin(fr)
    cos_t = np.concatenate([c64, c64], axis=0).astype(bf16)    # [128, S]
    sin_t = np.concatenate([s64, -s64], axis=0).astype(bf16)   # [128, S]
    u = np.arange(896, dtype=np.int64)[None, :]
    kvi = np.arange(128, dtype=np.int64)[:, None]
    stair = ((u - kvi) >= 384).astype(bf16)                # [128, 896]

    q_w = np.asarray(q_w, dtype=np.float32)
    k_w = np.asarray(k_w, dtype=np.float32)
    v_w = np.asarray(v_w, dtype=np.float32)
    o_w = np.asarray(o_w, dtype=np.float32)

    in_maps = []
    for c in range(NCORES):
        wqkv = np.ascontiguousarray(np.concatenate(
            [q_w[:, c * DQ:(c + 1) * DQ],
             k_w[:, c * HD:(c + 1) * HD],
             v_w[:, c * HD:(c + 1) * HD]], axis=1)).astype(bf16)
        # o_w rows for this core's 512 attn dims -> [d 128, head 4, hid 4096]
        owc = np.ascontiguousarray(
            o_w[c * DQ:(c + 1) * DQ, :].reshape(QH, HD, HIDDEN)
            .transpose(1, 0, 2)).astype(bf16)
        in_maps.append({"xT": xT, "wqkv": wqkv, "ow": owc,
                        "cos_t": cos_t, "sin_t": sin_t, "stair": stair})
    return in_maps


def run(inputs: dict, trace: bool = False):
    """Run on the 8 NeuronCores; returns (full_output, BassKernelResults)."""
    nc = build_program()
    in_maps = make_in_maps(**inputs)
    res = run_bass_kernel_spmd(nc, in_maps, core_ids=list(range(NCORES)),
                               trace=trace)
    acc = np.zeros((S, HIDDEN), dtype=np.float32)
    for c in range(NCORES):
        acc += np.asarray(res.results[c]["out"], dtype=np.float32)
    return acc.reshape(1, S, HIDDEN), res


def kernel(**inputs) -> np.ndarray:
    out, _ = run(inputs)
    return out
